# revision 17
# baseline (speedup 1.0000x reference)
"""Trainium2 Bass kernel for block-causal sparse attention (MLA-style KV).

Sharding: tensor-parallel over heads. 16 heads / 8 cores = 2 heads per core,
one KV head per core-pair. Each core computes q/k/v projections from the full
(transposed) x, RoPE, sparse attention for its 2 heads, and a partial output
projection; the host sums the 8 partial outputs.

Sparsity structure (T=4096, BLOCK=128, WINDOW=512, GLOBAL_EVERY=64):
for query block b, visible keys are blocks b-4..b (block b-4 masked by a fixed
triangular+global pattern) plus "global" columns j%64==0 with j < 128*(b-4).

v2 design notes (all-fp16 data path; PSUM accumulation stays fp32):
- fp16 matmuls run 1 cycle/row at any width (fp32r pays 4x below 256), and
  halve every DMA transfer and SBUF footprint.
- RoPE rotate-half is a partition swap; the head dims of q/k are permuted
  host-side (scores are invariant) so the pair partner is the adjacent
  partition, which DVE stream_shuffle can reach (32-lane quadrant-local).
- Softmax denominators come from summing the probability tiles on DVE and one
  gpsimd partition_all_reduce (which also yields the broadcast), freeing the
  PE of the ones-matmul reduction entirely. The 8 window items pack exactly
  into five full [128, 512] tiles: (b0), (b0-1), (b0-4 | b0+1),
  (b0-3 | b0+2), (b0-2 | b0+3).
- v transposes ride the XBAR DMA-transpose path (no PE/PSUM involvement).
- Software pipeline with one-iteration lag: iteration `it` runs
  projections(it), attention(it-1) and the output projection of it-2, so
  every cross-engine chain (rope, v-transpose, exp, normalize) has a full
  iteration of slack before the PE consumes its result.
"""

import numpy as np

N_CORES = 8
T = 4096
C = 2048
L = 512
H = 16
KVH = 4
HD = 128
BLOCK = 128
WINDOW = 512
GLOBAL_EVERY = 64
ROPE_THETA = 10000.0

QTW = 512            # query tile width (4 blocks)
NQT = T // QTW       # 8
NKT = C // 128       # 16 contraction tiles for projections
NG = T // GLOBAL_EVERY  # 64 global columns

_CACHE = {}

# stream_shuffle mask: swap adjacent partitions within each 32-lane quadrant
_SWAP_MASK = [i ^ 1 for i in range(32)]


def _build_module():
    import concourse.bacc as bacc
    import concourse.mybir as mybir
    import concourse.tile as tile
    import concourse.bass_isa as bass_isa
    from contextlib import ExitStack

    F16 = mybir.dt.float16
    F32 = mybir.dt.float32
    EXP = mybir.ActivationFunctionType.Exp
    RADD = bass_isa.ReduceOp.add

    nc = bacc.Bacc("TRN2", target_bir_lowering=False, debug=False,
                   num_devices=N_CORES)

    xt = nc.dram_tensor("xt", [128, NKT, T], F16, kind="ExternalInput")
    wall = nc.dram_tensor("wall", [128, NKT, 512], F16, kind="ExternalInput")
    wod = nc.dram_tensor("wod", [2 * HD, C], F16, kind="ExternalInput")
    cosd = nc.dram_tensor("cosd", [128, T], F16, kind="ExternalInput")
    sind = nc.dram_tensor("sind", [128, T], F16, kind="ExternalInput")  # sign-folded
    maskt = nc.dram_tensor("maskt", [128, 128], F16, kind="ExternalInput")
    maskg = nc.dram_tensor("maskg", [NG, T], F16, kind="ExternalInput")
    out = nc.dram_tensor("out", [T, C], F16, kind="ExternalOutput")

    scale = 1.0 / np.sqrt(HD)

    with tile.TileContext(nc) as tc, ExitStack() as ctx:
        res = ctx.enter_context(tc.tile_pool(name="res", bufs=1))
        kT = res.tile([128, T], F16, tag="kT")
        vN = res.tile([128, T], F16, tag="vN")
        kG = res.tile([128, NG], F16, tag="kG")
        vGT = res.tile([128, 128], F16, tag="vGT")
        vG = res.tile([128, 128], F16, tag="vG")
        mT = res.tile([128, 128], F16, tag="mT")
        mG = res.tile([NG, T], F16, tag="mG")
        wall_sb = res.tile([128, NKT, 512], F16, tag="wall_sb")
        wo_sb = res.tile([128, 2 * C], F16, tag="wo_sb")
        cos_sb = res.tile([128, T], F16, tag="cos_sb")
        sin_sb = res.tile([128, T], F16, tag="sin_sb")

        xpool = ctx.enter_context(tc.tile_pool(name="xpool", bufs=2))
        qlp = ctx.enter_context(tc.tile_pool(name="qlp", bufs=2))
        swp = ctx.enter_context(tc.tile_pool(name="swp", bufs=2))
        tmpp = ctx.enter_context(tc.tile_pool(name="tmpp", bufs=2))
        vtp = ctx.enter_context(tc.tile_pool(name="vtp", bufs=2))
        ppool = ctx.enter_context(tc.tile_pool(name="ppool", bufs=2))
        utp = ctx.enter_context(tc.tile_pool(name="utp", bufs=2))
        dbp = ctx.enter_context(tc.tile_pool(name="dbp", bufs=2))
        rcp = ctx.enter_context(tc.tile_pool(name="rcp", bufs=2))
        ynp = ctx.enter_context(tc.tile_pool(name="ynp", bufs=2))
        obp = ctx.enter_context(tc.tile_pool(name="obp", bufs=3))

        pjps = ctx.enter_context(tc.tile_pool(name="pjps", bufs=2, space="PSUM"))
        spool = ctx.enter_context(tc.tile_pool(name="spool", bufs=3, space="PSUM"))
        ypool = ctx.enter_context(tc.tile_pool(name="ypool", bufs=1, space="PSUM"))
        opool = ctx.enter_context(tc.tile_pool(name="opool", bufs=2, space="PSUM"))

        def emit_wo(ynorm, qs0, qs_list=(0, 1, 2, 3)):
            for qs in qs_list:
                rows = slice(qs0 + qs * 128, qs0 + (qs + 1) * 128)
                ob = obp.tile([128, C], F16, tag="ob", name="ob")
                for n in range(4):
                    o_ps = opool.tile([128, 512], F32, tag="o", name="o_ps")
                    nc.tensor.matmul(o_ps[:], ynorm[0][:, qs * 128:(qs + 1) * 128],
                                     wo_sb[:, n * 512:n * 512 + 512],
                                     start=True, stop=False)
                    nc.tensor.matmul(o_ps[:], ynorm[1][:, qs * 128:(qs + 1) * 128],
                                     wo_sb[:, C + n * 512:C + n * 512 + 512],
                                     start=False, stop=True)
                    if n == 3:
                        nc.vector.tensor_copy(ob[:, n * 512:(n + 1) * 512], o_ps[:])
                    else:
                        nc.scalar.copy(ob[:, n * 512:(n + 1) * 512], o_ps[:])
                nc.sync.dma_start(out[rows, :], ob[:])

        def emit_attention(it, qloc):
            """Attention for query tile `it` using the lagged pipeline state.
            Interleaves the deferred output projection between the heads."""
            nonlocal pending_wo
            b0 = 4 * it
            qs0 = it * QTW
            gw = min(NG, 8 * it)
            ynorm = []
            for h in range(2):
                if h == 1 and pending_wo is not None:
                    # first half here; second half lands after h1's PV
                    # matmuls so it overlaps the final normalize chain
                    emit_wo(*pending_wo, qs_list=(0, 1))
                packs = [("PA", [(b0, 0, 512)])]
                if it > 0:
                    packs.append(("PB", [(b0 - 1, 0, 512)]))
                packs.append(("PC", ([(b0 - 4, 0, 128)] if it > 0 else [])
                              + [(b0 + 1, 128, 384)]))
                packs.append(("PD", ([(b0 - 3, 0, 256)] if it > 0 else [])
                              + [(b0 + 2, 256, 256)]))
                packs.append(("PE", ([(b0 - 2, 0, 384)] if it > 0 else [])
                              + [(b0 + 3, 384, 128)]))
                use_glob = gw > 0
                if use_glob:
                    packs.append(("G", None))

                y_ps = ypool.tile([128, QTW], F32, tag="y")
                n_packs = len(packs)
                n_pv = sum(1 if p_[1] is None else len(p_[1]) for p_ in packs)
                s_tiles = [None] * n_packs
                p_tiles = [None] * n_packs
                pv_idx = [0]

                def emit_qk(ii):
                    tag, items = packs[ii]
                    s = spool.tile([128, QTW], F32, tag="s")
                    if items is None:
                        nc.tensor.matmul(s[:gw, :], kG[:, :gw], qloc[h][:],
                                         start=True, stop=True)
                    else:
                        for kb, qoff, w in items:
                            nc.tensor.matmul(
                                s[:, qoff:qoff + w],
                                kT[:, kb * 128:(kb + 1) * 128],
                                qloc[h][:, qoff:qoff + w],
                                start=True, stop=True)
                    s_tiles[ii] = s

                def emit_exp_pv(ii):
                    tag, items = packs[ii]
                    s = s_tiles[ii]
                    p = ppool.tile([128, QTW], F16, tag=tag, name=tag)
                    if items is None:
                        if it == 1:
                            # first generations of the G buffers: zero the
                            # rows above the (growing) written prefix once
                            nc.vector.memset(p[:], 0.0)
                        nc.scalar.activation(p[:gw, :], s[:gw, :], EXP, scale=scale)
                        nc.vector.tensor_mul(p[:gw, :], p[:gw, :],
                                             mG[:gw, qs0:qs0 + QTW])
                        nc.tensor.matmul(y_ps[:, :], vG[:gw, :], p[:gw, :],
                                         start=pv_idx[0] == 0,
                                         stop=pv_idx[0] == n_pv - 1)
                        pv_idx[0] += 1
                    else:
                        q_lo = min(qoff for _, qoff, _ in items)
                        if it == 0 and q_lo > 0:
                            # zero the never-written query columns of the
                            # first-generation pack buffers (it=0 only)
                            nc.vector.memset(p[:, 0:q_lo], 0.0)
                        nc.scalar.activation(p[:, q_lo:], s[:, q_lo:], EXP,
                                             scale=scale)
                        if it > 0 and tag == "PC":
                            nc.vector.tensor_mul(p[:, 0:128], p[:, 0:128], mT[:])
                        for kb, qoff, w in items:
                            nc.tensor.matmul(y_ps[:, qoff:qoff + w],
                                             vN[:, kb * 128:(kb + 1) * 128],
                                             p[:, qoff:qoff + w],
                                             start=pv_idx[0] == 0,
                                             stop=pv_idx[0] == n_pv - 1)
                            pv_idx[0] += 1
                    p_tiles[ii] = p

                def emit_usum(ii, acc):
                    # pairwise p-tile sums, emitted as packs complete
                    if ii == 1:
                        u = utp.tile([128, QTW], F16, tag="u01", name="u01")
                        nc.vector.tensor_add(u[:], p_tiles[0][:], p_tiles[1][:])
                        acc.append(u)
                    elif ii == 3:
                        u = utp.tile([128, QTW], F16, tag="u23", name="u23")
                        nc.vector.tensor_add(u[:], p_tiles[2][:], p_tiles[3][:])
                        ua = utp.tile([128, QTW], F16, tag="ua", name="ua")
                        nc.vector.tensor_add(ua[:], acc[0][:], u[:])
                        acc[0] = ua
                    elif ii == 5:
                        u = utp.tile([128, QTW], F16, tag="u45", name="u45")
                        nc.vector.tensor_add(u[:], p_tiles[4][:], p_tiles[5][:])
                        ub = utp.tile([128, QTW], F16, tag="ub", name="ub")
                        nc.vector.tensor_add(ub[:], acc[0][:], u[:])
                        acc[0] = ub

                emit_qk(0)
                if n_packs > 1:
                    emit_qk(1)
                acc = []
                for ii in range(n_packs):
                    if ii + 2 < n_packs:
                        emit_qk(ii + 2)
                    emit_exp_pv(ii)
                    emit_usum(ii, acc)

                dbc = dbp.tile([128, QTW], F32, tag="dbc")
                nc.gpsimd.partition_all_reduce(dbc[:], acc[0][:], channels=128,
                                               reduce_op=RADD)
                rec = rcp.tile([128, QTW], F32, tag="rec")
                nc.vector.reciprocal(rec[:], dbc[:])
                yn = ynp.tile([128, QTW], F16, tag=f"yn{h}", name=f"yn{h}")
                nc.vector.tensor_mul(yn[:], y_ps[:], rec[:])
                ynorm.append(yn)
            if pending_wo is not None:
                emit_wo(*pending_wo, qs_list=(2, 3))
                pending_wo = None
            return ynorm

        pending_wo = None
        pending_attn = None
        xq_tiles = {}

        def load_xq(it, chunked):
            xq = xpool.tile([128, NKT, 512], F16, tag="xq")
            ts = slice(it * 512, (it + 1) * 512)
            if chunked:
                for ck in range(4):
                    ks = slice(4 * ck, 4 * ck + 4)
                    nc.sync.dma_start(xq[:, ks, :], xt[:, ks, ts])
            else:
                nc.sync.dma_start(xq[:], xt[:, :, ts])
            xq_tiles[it] = xq

        for it in range(NQT):
            nt = it
            ts = slice(nt * 512, (nt + 1) * 512)

            # ---- loads (chunked + interleaved at startup; prefetched after)
            if it == 0:
                xq0 = xpool.tile([128, NKT, 512], F16, tag="xq", name="xq0")
                xq_tiles[0] = xq0
                for k0, k1 in ((0, 2), (2, 4), (4, 8), (8, 12), (12, 16)):
                    ks = slice(k0, k1)
                    nc.sync.dma_start(wall_sb[:, ks, :], wall[:, ks, :])
                    nc.sync.dma_start(xq0[:, ks, :], xt[:, ks, slice(0, 512)])
                nc.sync.dma_start(cos_sb[:, 0:512], cosd[:, 0:512])
                nc.sync.dma_start(sin_sb[:, 0:512], sind[:, 0:512])
                nc.gpsimd.dma_start(mT[:], maskt[:])
                nc.vector.memset(vGT[:], 0.0)
                load_xq(1, chunked=False)
                nc.sync.dma_start(cos_sb[:, 512:], cosd[:, 512:])
                nc.sync.dma_start(sin_sb[:, 512:], sind[:, 512:])
                nc.gpsimd.dma_start(mG[:], maskg[:])
                for i in range(2):
                    nc.sync.dma_start(wo_sb[:, i * C:(i + 1) * C],
                                      wod[i * 128:(i + 1) * 128, :])
            elif it + 1 < NQT:
                load_xq(it + 1, chunked=False)
            xq = xq_tiles.pop(it)

            cos_t = cos_sb[:, ts]
            sin_t = sin_sb[:, ts]

            # ---- projections in two passes of two columns (q0+q1, k+v) ----
            qloc = [qlp.tile([128, 512], F16, tag=f"ql{h}", name=f"ql{h}")
                    for h in range(2)]
            vsb = vtp.tile([128, 512], F16, tag="vsb")

            def rope(pj, dest):
                qsb = swp.tile([128, 512], F16, tag="qsb")
                nc.scalar.copy(qsb[:], pj[:])
                sw = swp.tile([128, 512], F16, tag="sw")
                nc.vector.stream_shuffle(sw[:], qsb[:], _SWAP_MASK)
                ta = tmpp.tile([128, 512], F16, tag="ta")
                nc.vector.tensor_mul(ta[:], qsb[:], cos_t)
                tb = tmpp.tile([128, 512], F16, tag="tb")
                nc.vector.tensor_mul(tb[:], sw[:], sin_t)
                nc.vector.tensor_add(dest, ta[:], tb[:])

            if it == 0:
                # attention psum is idle on the first tile: run all four
                # projection columns in one pass (2 pjps + 2 borrowed spool
                # banks) so each x chunk is consumed as soon as it lands
                pj4 = [pjps.tile([128, 512], F32, tag="pj", name=f"pj4{i}")
                       for i in range(2)]
                pj4 += [spool.tile([128, QTW], F32, tag="s", name=f"pj4{i + 2}")
                        for i in range(2)]
                for kt in range(NKT):
                    for col in range(4):
                        nc.tensor.matmul(pj4[col][:],
                                         wall_sb[:, kt, col * 128:(col + 1) * 128],
                                         xq[:, kt, :],
                                         start=(kt == 0), stop=(kt == NKT - 1))
                rope(pj4[0], qloc[0][:])
                rope(pj4[1], qloc[1][:])
                rope(pj4[2], kT[:, ts])
                nc.vector.tensor_copy(vsb[:], pj4[3][:])
            else:
                for pair in range(2):
                    pjs = [pjps.tile([128, 512], F32, tag="pj", name=f"pj{pair}{i}")
                           for i in range(2)]
                    for kt in range(NKT):
                        for i in range(2):
                            col = pair * 2 + i
                            nc.tensor.matmul(pjs[i][:],
                                             wall_sb[:, kt, col * 128:(col + 1) * 128],
                                             xq[:, kt, :],
                                             start=(kt == 0), stop=(kt == NKT - 1))
                    if pair == 0:
                        rope(pjs[0], qloc[0][:])
                        rope(pjs[1], qloc[1][:])
                    else:
                        rope(pjs[0], kT[:, ts])
                        nc.vector.tensor_copy(vsb[:], pjs[1][:])

            # ---- lagged attention for tile it-1 (incl. deferred wo) ----
            if pending_attn is not None:
                ait, aqloc = pending_attn
                ynorm = emit_attention(ait, aqloc)
                pending_wo = (ynorm, ait * QTW)
            pending_attn = (it, qloc)

            # ---- v transpose into vN (xbar DMA) + incremental global K/V ----
            for j in range(4):
                blk = nt * 4 + j
                nc.sync.dma_start_transpose(vN[:, blk * 128:(blk + 1) * 128],
                                            vsb[:, j * 128:(j + 1) * 128])
            gsl = slice(nt * 8, (nt + 1) * 8)
            nc.vector.tensor_copy(kG[:, gsl], kT[:, ts][:, 0:512:GLOBAL_EVERY])
            nc.vector.tensor_copy(vGT[:, gsl], vsb[:, 0:512:GLOBAL_EVERY])
            nc.sync.dma_start_transpose(vG[:], vGT[:])

        # ---- tail: attention for the last tile + remaining projections ----
        ait, aqloc = pending_attn
        ynorm = emit_attention(ait, aqloc)
        emit_wo(ynorm, ait * QTW)

    nc.compile()
    return nc


def _host_inputs(x, w_q, w_kv_down, w_k_up, w_v_up, w_o):
    """Build the per-core input maps (host-side shard + precompute)."""
    x = np.asarray(x)
    w_q = np.asarray(w_q)
    w_kv_down = np.asarray(w_kv_down)
    w_k_up = np.asarray(w_k_up)
    w_v_up = np.asarray(w_v_up)
    w_o = np.asarray(w_o)
    x2 = x.reshape(T, C).astype(np.float32)
    # x transposed, grouped for one-DMA tile loads: xt[p, kt, t]
    xt = np.ascontiguousarray(
        x2.T.reshape(NKT, 128, T).transpose(1, 0, 2).astype(np.float16))

    # head-dim pair permutation: new dim 2d <- d, 2d+1 <- d+64
    perm = np.empty(HD, np.int64)
    perm[0::2] = np.arange(64)
    perm[1::2] = np.arange(64, 128)

    # RoPE tables in permuted order, [128, T]; sign folded into sin
    freqs = 1.0 / (ROPE_THETA ** (np.arange(0, HD, 2, dtype=np.float64) / HD))
    emb = np.arange(T, dtype=np.float64)[:, None] * freqs[None, :]   # [T, 64]
    cosP = np.empty((HD, T), np.float64)
    sinP = np.empty((HD, T), np.float64)
    cosP[0::2] = cosP[1::2] = np.cos(emb).T
    sinP[0::2] = -np.sin(emb).T
    sinP[1::2] = np.sin(emb).T
    cosP = np.ascontiguousarray(cosP.astype(np.float16))
    sinP = np.ascontiguousarray(sinP.astype(np.float16))

    # fixed triangular+global mask for the b-4 key block, [k_off, q_off]
    oi = np.arange(128)
    mT = ((oi[None, :] <= oi[:, None]) | (oi[:, None] % 64 == 0)).astype(np.float16)

    # global-column mask [g, q]: visible iff 64 g < 128 (q//128 - 4)
    g = np.arange(NG)
    qb = np.arange(T) // BLOCK
    mG = (64 * g[:, None] < 128 * (qb[None, :] - 4)).astype(np.float16)

    wk_f = (w_kv_down.astype(np.float32) @ w_k_up.astype(np.float32))  # [C, KVH*HD]
    wv_f = (w_kv_down.astype(np.float32) @ w_v_up.astype(np.float32))

    in_maps = []
    for c in range(N_CORES):
        h0 = 2 * c
        kv = h0 // (H // KVH)
        wq_c = w_q[:, h0 * HD:(h0 + 2) * HD].astype(np.float32)
        wk_c = wk_f[:, kv * HD:(kv + 1) * HD]
        wv_c = wv_f[:, kv * HD:(kv + 1) * HD]
        # apply the rope pair permutation to q/k projection columns
        wq_p = wq_c.reshape(C, 2, HD)[:, :, perm].reshape(C, 2 * HD)
        wk_p = wk_c[:, perm]
        # pack [q0 q1 k v] -> [C, 512] -> wall[p, kt, col]
        wcat = np.concatenate([wq_p, wk_p, wv_c], axis=1)          # [C, 512]
        wall = np.ascontiguousarray(
            wcat.reshape(NKT, 128, 512).transpose(1, 0, 2).astype(np.float16))
        wo_c = np.ascontiguousarray(
            w_o[h0 * HD:(h0 + 2) * HD, :].astype(np.float16))
        in_maps.append({
            "xt": xt, "wall": wall, "wod": wo_c,
            "cosd": cosP, "sind": sinP, "maskt": mT, "maskg": mG,
        })
    return in_maps


def _get_module():
    if "nc" not in _CACHE:
        _CACHE["nc"] = _build_module()
    return _CACHE["nc"]


def kernel(x, w_q, w_kv_down, w_k_up, w_v_up, w_o):
    from concourse.bass_utils import run_bass_kernel_spmd

    nc = _get_module()
    in_maps = _host_inputs(x, w_q, w_kv_down, w_k_up, w_v_up, w_o)
    res = run_bass_kernel_spmd(nc, in_maps, list(range(N_CORES)))
    acc = np.zeros((T, C), np.float32)
    for c in range(N_CORES):
        acc += res.results[c]["out"].astype(np.float32)
    return acc.reshape(1, T, C)
